# revision 6
# baseline (speedup 1.0000x reference)
"""DecoderTreeRNN Trainium2 kernel (8 NeuronCores, single SPMD launch).

  - Tree expansion: data-parallel over batch B (8 examples/core). GRU states
    kept transposed [H, nodes] in bf16; each level is ghT = WhhT.T @ hT with
    fp8(e4m3) weight tiles stationary on the PE (fp8 FWL makes the weight
    load, the tree's floor, 2-4x faster; states stay bf16). Gate biases are
    folded in with free-dim-broadcast adds on VectorE; sigmoid/tanh run on
    ScalarE from one ACT table set, all on 4-wide m-tile slabs. Children are
    concatenated [left | right]; the bit-reversed leaf order is undone on
    the host during unshard. The last level writes fp8 states directly.
  - The fp8 leaf states are AllGathered so every core holds all B*32 rows.
  - Output projection: tensor-parallel over vocab (4000 columns/core),
    fp8 DoubleRow matmuls (K=256 per tile, pre-paired k=256*k2+128*j+p
    layout on both operands). Per row tile one stationary leaf tile serves
    all 8 vocab chunks, each accumulating in its own PSUM bank. The f32
    vocab bias is added during the PSUM->SBUF copy (VectorE) and exp +
    row-sum is fused on ScalarE via accum_out. Unnormalized logits stream
    straight out; each core also returns its per-row exp-sums and
    -log(sum over cores) is folded into the host-side unshard pass.
  DMA discipline: the two HWDGE rings (SP + ACT) are ordered FIFOs - small
  latency-critical inputs and right-side tree weights on the ACT ring,
  left-side tree weights then projection weights on the SP ring.
"""

import sys

for _p in ("/opt/trn_rl_repo",):
    if _p not in sys.path:
        sys.path.append(_p)

import numpy as np
import ml_dtypes

import concourse.bass as bass
from concourse import bacc, tile, mybir
from concourse import bass_utils
from concourse.tile_rust import add_dep_helper
from contextlib import ExitStack

BF16 = mybir.dt.bfloat16
F32 = mybir.dt.float32
AF = mybir.ActivationFunctionType
ALU = mybir.AluOpType
BFNP = ml_dtypes.bfloat16
FP8 = mybir.dt.float8e4
FP8_AG = True   # leaves in fp8: feeds the DoubleRow projection

N_CORES = 8
CW = 500  # vocab chunk width (<=512 fp32 psum bank)


def _build(B, H, V, DEPTH):
    KT = H // 128            # contraction tiles
    MT = 3 * KT              # output m-tiles per GRU side
    Bl = B // N_CORES        # examples per core
    L = 1 << DEPTH           # leaves per example
    NLOC = Bl * L            # local leaf count
    ROWS = B * L             # total leaf rows
    RT = ROWS // 128         # row tiles
    Vs = V // N_CORES        # vocab shard
    NCH = Vs // CW           # chunks per shard
    SG = min(4, KT)          # m-tiles per gate slab
    NSL = KT // SG           # slabs per gate
    assert B % N_CORES == 0 and H % 128 == 0 and V % N_CORES == 0
    assert Vs % CW == 0 and ROWS % 128 == 0 and RT <= 512
    assert SG * 128 <= 512  # psum slab fits one bank

    nc = bacc.Bacc("TRN2", target_bir_lowering=False, debug=False,
                   num_devices=N_CORES, dynamic_dma_scratch_size=2048)

    # ---------------- DRAM I/O ----------------
    encT = nc.dram_tensor("encT", [H, Bl], BF16, kind="ExternalInput")
    wt_d, wb_d, bih2_d = {}, {}, {}
    for s in "lr":
        wt_d[s] = nc.dram_tensor(f"wt_{s}", [H, 3 * H], FP8, kind="ExternalInput")
        wb_d[s] = nc.dram_tensor(f"wb_{s}", [128, 3 * KT], F32, kind="ExternalInput")
        bih2_d[s] = nc.dram_tensor(f"bih2_{s}", [128, KT], F32,
                                   kind="ExternalInput")
    KT2 = KT // 2            # DoubleRow k-tiles (K=256 each)
    woT_d = nc.dram_tensor("woT", [128, KT2, 2, Vs], FP8, kind="ExternalInput")
    bo_d = nc.dram_tensor("bo", [128, Vs], F32, kind="ExternalInput")
    out_d = nc.dram_tensor("out", [ROWS, Vs], F32, kind="ExternalOutput")

    AGDT = FP8 if FP8_AG else BF16
    ag_leaves = nc.dram_tensor("ag_leaves", [N_CORES * H, NLOC], AGDT,
                               kind="Internal", addr_space="Shared")
    s_out_d = nc.dram_tensor("s_out", [128, RT], F32, kind="ExternalOutput")

    rg = [list(range(N_CORES))]

    with tile.TileContext(nc) as tc, ExitStack() as ctx:
        dram = ctx.enter_context(tc.tile_pool(name="dram", bufs=1, space="DRAM"))
        wproj = ctx.enter_context(tc.tile_pool(name="wproj", bufs=1))
        cpool = ctx.enter_context(tc.tile_pool(name="const", bufs=1))

        # projection weights: resident for the whole kernel. Tiles are
        # allocated up front but their DMAs are issued after the tree weight
        # DMAs (below) so the tree isn't starved of HBM bandwidth at start.
        wo_sb = wproj.tile([128, KT2, 2, Vs], FP8, tag="wo8", name="wo8")
        bo_sb = cpool.tile([128, Vs], F32, tag="bo")
        ones_sb = cpool.tile([1, 128], BF16, tag="ones")
        nc.vector.memset(ones_sb[:], 1.0)

        leaves_bounce = dram.tile([H, NLOC], AGDT, tag="lvb")

        # ---------------- tree expansion ----------------
        with nc.named_scope("tree"):
            with tc.tile_pool(name="wtree", bufs=1) as wtp, \
                 tc.tile_pool(name="state", bufs=2) as stp, \
                 tc.tile_pool(name="gates", bufs=2) as gp, \
                 tc.tile_pool(name="pstree", bufs=8, space="PSUM") as pst:
                # latency-critical small inputs go on the ACT HWDGE ring so
                # they aren't stuck behind the big weight loads (SP ring FIFO)
                cur = stp.tile([128, KT, Bl], BF16, tag="st")
                nc.scalar.dma_start(cur[:], encT.ap().rearrange("(k p) b -> p k b", k=KT))
                wt_sb, wb_sb, bih2_sb = {}, {}, {}
                for s in "lr":
                    wb_sb[s] = wtp.tile([128, 3 * KT], F32, tag=f"wb{s}", name=f"wb_sb_{s}")
                    nc.scalar.dma_start(wb_sb[s][:], wb_d[s].ap())
                    bih2_sb[s] = wtp.tile([128, KT], F32, tag=f"bi{s}", name=f"bih2_sb_{s}")
                    nc.scalar.dma_start(bih2_sb[s][:], bih2_d[s].ap())
                # weight loads in consumption order: side l, side r, then the
                # projection weights behind them (all FIFO on the SP ring)
                for s in "lr":
                    eng = nc.sync if s == "l" else nc.scalar
                    wt_sb[s] = []
                    for k in range(KT):
                        t = wtp.tile([128, 3 * H], FP8, tag=f"wt{s}{k}")
                        eng.dma_start(t[:], wt_d[s].ap()[128 * k:128 * (k + 1), :])
                        wt_sb[s].append(t)
                nc.sync.dma_start(wo_sb[:], woT_d.ap())
                nc.sync.dma_start(bo_sb[:], bo_d.ap())

                n = Bl
                for lvl in range(DEPTH):
                    last = lvl == DEPTH - 1
                    nxt = stp.tile([128, KT, 2 * n], AGDT if last else BF16,
                                   tag="st8" if last else "st",
                                   name=f"nxt{lvl}", bufs=1 if last else None)
                    for si, s in enumerate("lr"):
                        for sl in range(NSL):
                            ko0 = sl * SG
                            ps = {}
                            for gi, mb in (("r", ko0), ("z", KT + ko0), ("g", 2 * KT + ko0)):
                                p = pst.tile([128, SG, n], F32, tag="ps")
                                for mj in range(SG):
                                    m = mb + mj
                                    for k in range(KT):
                                        nc.tensor.matmul(
                                            p[:, mj, :],
                                            wt_sb[s][k][:, 128 * m:128 * (m + 1)],
                                            cur[:, k, :n],
                                            start=(k == 0), stop=(k == KT - 1))
                                ps[gi] = p
                            # biases folded in via free-dim-broadcast adds (DVE)
                            def _bias(mb_):
                                return wb_sb[s][:, mb_:mb_ + SG].unsqueeze(2)                                    .broadcast_to((128, SG, n))
                            y_r = gp.tile([128, SG, n], F32, tag="yr")
                            nc.vector.tensor_tensor(y_r[:], ps["r"][:], _bias(ko0), op=ALU.add)
                            r_t = gp.tile([128, SG, n], F32, tag="r")
                            nc.scalar.activation(r_t[:], y_r[:], AF.Sigmoid)
                            y_z = gp.tile([128, SG, n], F32, tag="yz")
                            nc.vector.tensor_tensor(y_z[:], ps["z"][:], _bias(KT + ko0), op=ALU.add)
                            z_t = gp.tile([128, SG, n], F32, tag="z")
                            nc.scalar.activation(z_t[:], y_z[:], AF.Sigmoid)
                            y_g = gp.tile([128, SG, n], F32, tag="yg")
                            nc.vector.tensor_tensor(y_g[:], ps["g"][:], _bias(2 * KT + ko0), op=ALU.add)
                            t_t = gp.tile([128, SG, n], F32, tag="t")
                            nc.vector.tensor_tensor(t_t[:], y_g[:], r_t[:], op=ALU.mult)
                            nc.vector.tensor_tensor(
                                t_t[:], t_t[:],
                                bih2_sb[s][:, ko0:ko0 + SG].unsqueeze(2)
                                .broadcast_to((128, SG, n)), op=ALU.add)
                            n_t = gp.tile([128, SG, n], F32, tag="n")
                            nc.scalar.activation(n_t[:], t_t[:], AF.Tanh)
                            u_t = gp.tile([128, SG, n], F32, tag="u")
                            nc.vector.scalar_tensor_tensor(
                                u_t[:], n_t[:], -1.0, cur[:, ko0:ko0 + SG, :n],
                                op0=ALU.mult, op1=ALU.add)  # u = h - n
                            nc.vector.tensor_tensor(u_t[:], u_t[:], z_t[:], op=ALU.mult)
                            nc.vector.tensor_tensor(
                                nxt[:, ko0:ko0 + SG, si * n:si * n + n],
                                u_t[:], n_t[:], op=ALU.add)
                    cur = nxt
                    n *= 2

                for k in range(KT):
                    eng = nc.sync if k % 2 == 0 else nc.scalar
                    eng.dma_start(leaves_bounce[128 * k:128 * (k + 1), :],
                                  cur[:, k, :])

        # ---------------- leaves all-gather ----------------
        with nc.named_scope("ag_leaves"):
            nc.gpsimd.collective_compute(
                "AllGather", ALU.bypass, replica_groups=rg,
                ins=[leaves_bounce.opt()], outs=[ag_leaves.ap()])

        # ---------------- projection + log-softmax ----------------
        with nc.named_scope("proj"):
            with tc.tile_pool(name="leaves", bufs=1) as lvp, \
                 tc.tile_pool(name="logits", bufs=3) as lgp, \
                 tc.tile_pool(name="scr", bufs=4) as scp, \
                 tc.tile_pool(name="stats", bufs=2) as sp2, \
                 tc.tile_pool(name="psproj", bufs=8, space="PSUM") as psp:
                ag_view = ag_leaves.ap().rearrange("(c h) j -> h c j", c=N_CORES)
                lvbig = lvp.tile([128, KT, N_CORES * NLOC], AGDT, tag="lvbig")
                for k in range(KT):
                    eng = nc.sync if k % 2 == 0 else nc.scalar
                    eng.dma_start(
                        lvbig[:, k, :].rearrange("p (c j) -> p c j", c=N_CORES),
                        ag_view[128 * k:128 * (k + 1)])

                # unnormalized logits stream out as soon as each row tile is
                # done; the per-shard softmax denominators are returned as a
                # tiny second output and log(sum) is folded into the host-side
                # unshard pass.
                s_all = sp2.tile([128, RT], F32, tag="sall", name="s_all")
                # nosync chain: keep the PE stream in emission order
                # (k2-outer, nch-inner) so all NCH matmuls sharing a
                # stationary tile stay adjacent; _dedup_ldweights then strips
                # the redundant stationary reloads legalize inserts.
                pe_prev = None
                for r in range(RT):
                    lg = lgp.tile([128, Vs], F32, tag="lg", name=f"lg{r}")
                    sp = sp2.tile([128, NCH], F32, tag="spart", name=f"sp{r}")
                    # k-outer so one stationary (leaves) tile serves all NCH
                    # chunks; each chunk accumulates in its own PSUM bank
                    pps = [psp.tile([128, CW], F32, tag="pp", name=f"pp{r}_{nch}")
                           for nch in range(NCH)]
                    for k2 in range(KT2):
                        lhsT = lvbig[:, 2 * k2:2 * k2 + 2, 128 * r:128 * (r + 1)]
                        for nch in range(NCH):
                            mm = nc.tensor.matmul(
                                pps[nch][:], lhsT,
                                wo_sb[:, k2, :, CW * nch:CW * (nch + 1)],
                                perf_mode=mybir.MatmulPerfMode.DoubleRow,
                                start=(k2 == 0), stop=(k2 == KT2 - 1))
                            if pe_prev is not None:
                                add_dep_helper(mm.ins, pe_prev, sync=False,
                                               reason="proj pe order")
                            pe_prev = mm.ins
                    for nch in range(NCH):
                        # bias add fused into the PSUM->SBUF copy
                        nc.vector.tensor_tensor(
                            lg[:, CW * nch:CW * (nch + 1)], pps[nch][:],
                            bo_sb[:, CW * nch:CW * (nch + 1)],
                            op=ALU.add)
                        ex = scp.tile([128, CW], BF16, tag="exp",
                                      name=f"ex{r}_{nch}")
                        nc.scalar.activation(ex[:],
                                             lg[:, CW * nch:CW * (nch + 1)],
                                             AF.Exp,
                                             accum_out=sp[:, nch:nch + 1])
                    nc.vector.reduce_sum(s_all[:, r:r + 1], sp[:],
                                         axis=mybir.AxisListType.X)
                    nc.sync.dma_start(out_d.ap()[128 * r:128 * (r + 1), :], lg[:])
                nc.scalar.dma_start(s_out_d.ap()[:, :], s_all[:])

    _dedup_ldweights(nc)
    nc.compile()
    return nc


def _dedup_ldweights(nc):
    """Remove consecutive duplicate InstLdweights from the scheduled PE
    stream: the PE keeps the stationary operand loaded across matmuls, so a
    reload of the identical weights AP between non-self-loading matmuls is
    pure overhead (tile_legalize emits one per matmul unconditionally)."""
    removed = 0
    for fn in nc.m.functions:
        for blk in fn.blocks:
            prev = None   # (ap_str, perf_mode, tile_position, is_transpose)
            keep = []
            for inst in blk.instructions:
                if isinstance(inst, mybir.InstLdweights):
                    si = inst.sync_info
                    clean = si is None or (not si.on_wait and not si.on_update)
                    key = (str(inst.ins[0]), str(inst.perf_mode),
                           str(inst.tile_position), str(inst.is_transpose))
                    if clean and key == prev:
                        removed += 1
                        continue
                    prev = key
                elif isinstance(inst, mybir.InstMatmult):
                    # non-self-loading matmuls leave the stationary intact;
                    # anything else (self-loading, transpose) invalidates it
                    if inst.ldweights is not False:
                        prev = None
                elif inst.engine == mybir.EngineType.PE and inst.is_executable():
                    prev = None
                keep.append(inst)
            if removed:
                blk.instructions[:] = keep
    return removed


_CACHE = {}


def _get(B, H, V, DEPTH):
    key = (B, H, V, DEPTH)
    if key not in _CACHE:
        _CACHE[key] = _build(B, H, V, DEPTH)
    return _CACHE[key]


def _pack_inputs(B, H, V, DEPTH, encoding, Whh_l, bih_l, bhh_l, Whh_r, bih_r,
                 bhh_r, W_out, b_out):
    """Host-side shard + transpose + cast. Returns in_maps for the 8 cores."""
    KT = H // 128
    Bl = B // N_CORES
    Vs = V // N_CORES

    KT2 = KT // 2
    woT = np.ascontiguousarray(W_out.T).astype(np.float32)    # [H, V]
    encT = np.ascontiguousarray(encoding.T).astype(BFNP)      # [H, B]

    shared = {}
    for s, Whh, bih, bhh in (("l", Whh_l, bih_l, bhh_l), ("r", Whh_r, bih_r, bhh_r)):
        shared[f"wt_{s}"] = np.ascontiguousarray(Whh.T).astype(
            mybir.dt.np(FP8))  # [H, 3H] fp8: weight-load bound, not precision bound
        # bias row folded into the matmul: sigmoid gates get bih+bhh,
        # candidate gate gets bhh only (bih_n is added after the r-multiply)
        wb = np.concatenate([(bih + bhh)[:2 * H], bhh[2 * H:]])
        shared[f"wb_{s}"] = np.ascontiguousarray(
            wb.reshape(3 * KT, 128).T.astype(np.float32))
        shared[f"bih2_{s}"] = np.ascontiguousarray(
            bih[2 * H:].reshape(KT, 128).T.astype(np.float32))  # [128, KT]

    in_maps = []
    for c in range(N_CORES):
        m = dict(shared)
        m["encT"] = np.ascontiguousarray(encT[:, c * Bl:(c + 1) * Bl])
        w = woT[:, c * Vs:(c + 1) * Vs].reshape(KT2, 2, 128, Vs)
        m["woT"] = np.ascontiguousarray(
            w.transpose(2, 0, 1, 3)).astype(mybir.dt.np(FP8))
        m["bo"] = np.ascontiguousarray(np.broadcast_to(
            b_out[c * Vs:(c + 1) * Vs].astype(np.float32), (128, Vs)))
        in_maps.append(m)
    return in_maps


def _run(B, H, V, DEPTH, inputs, trace=False, nc=None):
    if nc is None:
        nc = _get(B, H, V, DEPTH)
    in_maps = _pack_inputs(B, H, V, DEPTH, **inputs)
    res = bass_utils.run_bass_kernel_spmd(
        nc, in_maps, core_ids=list(range(N_CORES)), trace=trace)

    L = 1 << DEPTH
    Bl = B // N_CORES
    Vs = V // N_CORES
    # leaf column order per core: col = jj*Bl + e with jj = bitrev(true leaf)
    rev = np.array([int(format(t, f"0{DEPTH}b")[::-1], 2) for t in range(L)])
    # log-softmax denominator: sum the per-shard exp-sums across cores
    s_tot = np.zeros((B * L,), np.float64)
    for c in range(N_CORES):
        s = res.results[c]["s_out"]                  # [128, RT]
        s_tot += s.T.reshape(-1).astype(np.float64)  # row = rt*128 + p
    lse = np.log(s_tot).astype(np.float32)           # [B*L] in device row order
    lse = lse.reshape(N_CORES, L, Bl).transpose(0, 2, 1).reshape(B, L)[:, rev]
    full = np.empty((B, L, V), np.float32)
    for c in range(N_CORES):
        o = res.results[c]["out"]                    # [B*L, Vs]
        o = o.reshape(N_CORES, L, Bl, Vs)            # [src_core, jj, e, v]
        o = o.transpose(0, 2, 1, 3).reshape(B, L, Vs)
        full[:, :, c * Vs:(c + 1) * Vs] = o[:, rev, :] - lse[:, :, None]
    return full, res


def kernel(**inputs):
    enc = np.asarray(inputs["encoding"], np.float32)
    B, H = enc.shape
    V = np.asarray(inputs["W_out"]).shape[0]
    DEPTH = int(inputs["depth"])
    args = {k: np.asarray(v, np.float32) for k, v in inputs.items() if k != "depth"}
    full, _ = _run(B, H, V, DEPTH, args)
    return full



# revision 18
# speedup vs baseline: 1.1185x; 1.1185x over previous
"""DecoderTreeRNN Trainium2 kernel (8 NeuronCores, single SPMD launch).

  - Tree expansion: data-parallel over batch B (8 examples/core). GRU states
    kept transposed [H, nodes] in bf16; each level is ghT = WhhT.T @ hT with
    fp8(e4m3) weight tiles stationary on the PE (fp8 FWL makes the weight
    load, the tree's floor, 2-4x faster; states stay bf16). Gate biases are
    folded in with free-dim-broadcast adds on VectorE; sigmoid/tanh run on
    ScalarE from one ACT table set, all on 4-wide m-tile slabs. Children are
    concatenated [left | right]; the bit-reversed leaf order is undone on
    the host during unshard. The last level writes fp8 states directly.
  - The fp8 leaf states are AllGathered so every core holds all B*32 rows.
  - Output projection: tensor-parallel over vocab (4000 columns/core),
    fp8 DoubleRow matmuls (K=256 per tile, pre-paired k=256*k2+128*j+p
    layout on both operands). Per row tile one stationary leaf tile serves
    all 8 vocab chunks, each accumulating in its own PSUM bank. The f32
    vocab bias is added during the PSUM->SBUF copy (VectorE) and exp +
    row-sum is fused on ScalarE via accum_out. Unnormalized logits stream
    straight out; each core also returns its per-row exp-sums and
    -log(sum over cores) is folded into the host-side unshard pass.
  DMA discipline: the two HWDGE rings (SP + ACT) are ordered FIFOs - small
  latency-critical inputs and right-side tree weights on the ACT ring,
  left-side tree weights then projection weights on the SP ring.
"""

import sys

for _p in ("/opt/trn_rl_repo",):
    if _p not in sys.path:
        sys.path.append(_p)

import numpy as np
import ml_dtypes

import concourse.bass as bass
from concourse import bacc, tile, mybir
from concourse import bass_utils
from concourse.tile_rust import add_dep_helper
from contextlib import ExitStack

BF16 = mybir.dt.bfloat16
F32 = mybir.dt.float32
AF = mybir.ActivationFunctionType
ALU = mybir.AluOpType
BFNP = ml_dtypes.bfloat16
FP8 = mybir.dt.float8e4
FP8_AG = True   # leaves in fp8: feeds the DoubleRow projection

N_CORES = 8
CW = 500  # vocab chunk width (<=512 fp32 psum bank)


def _build(B, H, V, DEPTH):
    KT = H // 128            # contraction tiles
    MT = 3 * KT              # output m-tiles per GRU side
    Bl = B // N_CORES        # examples per core
    L = 1 << DEPTH           # leaves per example
    NLOC = Bl * L            # local leaf count
    ROWS = B * L             # total leaf rows
    RT = ROWS // 128         # row tiles
    Vs = V // N_CORES        # vocab shard
    NCH = Vs // CW           # chunks per shard
    SG = min(4, KT)          # m-tiles per gate slab
    NSL = KT // SG           # slabs per gate
    assert B % N_CORES == 0 and H % 128 == 0 and V % N_CORES == 0
    assert Vs % CW == 0 and ROWS % 128 == 0 and RT <= 512
    assert SG * 128 <= 512  # psum slab fits one bank

    nc = bacc.Bacc("TRN2", target_bir_lowering=False, debug=False,
                   num_devices=N_CORES, dynamic_dma_scratch_size=2048)

    # ---------------- DRAM I/O ----------------
    encT = nc.dram_tensor("encT", [H, Bl], BF16, kind="ExternalInput")
    wt_d, wb_d, bih2_d = {}, {}, {}
    for s in "lr":
        wt_d[s] = nc.dram_tensor(f"wt_{s}", [H, 3 * H], FP8, kind="ExternalInput")
        wb_d[s] = nc.dram_tensor(f"wb_{s}", [128, 3 * KT], F32, kind="ExternalInput")
        bih2_d[s] = nc.dram_tensor(f"bih2_{s}", [128, KT], F32,
                                   kind="ExternalInput")
    KT2 = KT // 2            # DoubleRow k-tiles (K=256 each)
    woT_d = nc.dram_tensor("woT", [128, KT2, 2, Vs], FP8, kind="ExternalInput")
    bo_d = nc.dram_tensor("bo", [128, Vs], BF16, kind="ExternalInput")
    out_d = nc.dram_tensor("out", [ROWS, Vs], BF16, kind="ExternalOutput")

    AGDT = FP8 if FP8_AG else BF16
    # leaves are exchanged in two halves (side l, then side r) so the first
    # AllGather overlaps the second half of level-4 compute and the second
    # overlaps projection on the first half's rows. Layout [128, KT*128]
    # keeps every DMA 1KB-contiguous per partition.
    ag_half = [nc.dram_tensor(f"ag_lv{si}", [N_CORES, 128, KT * 128], AGDT,
                              kind="Internal", addr_space="Shared")
               for si in range(2)]
    s_out_d = nc.dram_tensor("s_out", [128, RT], F32, kind="ExternalOutput")

    rg = [list(range(N_CORES))]

    with tile.TileContext(nc) as tc, ExitStack() as ctx:
        dram = ctx.enter_context(tc.tile_pool(name="dram", bufs=1, space="DRAM"))
        wproj = ctx.enter_context(tc.tile_pool(name="wproj", bufs=1))
        cpool = ctx.enter_context(tc.tile_pool(name="const", bufs=1))

        # projection weights: resident for the whole kernel. Tiles are
        # allocated up front but their DMAs are issued after the tree weight
        # DMAs (below) so the tree isn't starved of HBM bandwidth at start.
        wo_sb = wproj.tile([128, KT2, 2, Vs], FP8, tag="wo8", name="wo8")
        bo_sb = cpool.tile([128, Vs], BF16, tag="bo")
        ones_sb = cpool.tile([1, 128], BF16, tag="ones")
        nc.vector.memset(ones_sb[:], 1.0)

        lvb = [dram.tile([128, KT * 128], AGDT, tag=f"lvb{si}", name=f"lvb{si}")
               for si in range(2)]

        # ---------------- tree expansion ----------------
        with nc.named_scope("tree"):
            with tc.tile_pool(name="wtree", bufs=1) as wtp, \
                 tc.tile_pool(name="state", bufs=2) as stp, \
                 tc.tile_pool(name="gates", bufs=2) as gp, \
                 tc.tile_pool(name="pstree", bufs=8, space="PSUM") as pst:
                # latency-critical small inputs go on the ACT HWDGE ring so
                # they aren't stuck behind the big weight loads (SP ring FIFO)
                cur = stp.tile([128, KT, Bl], BF16, tag="st")
                nc.scalar.dma_start(cur[:], encT.ap().rearrange("(k p) b -> p k b", k=KT))
                wt_sb, wb_sb, bih2_sb = {}, {}, {}
                for s in "lr":
                    wb_sb[s] = wtp.tile([128, 3 * KT], F32, tag=f"wb{s}", name=f"wb_sb_{s}")
                    nc.scalar.dma_start(wb_sb[s][:], wb_d[s].ap())
                    bih2_sb[s] = wtp.tile([128, KT], F32, tag=f"bi{s}", name=f"bih2_sb_{s}")
                    nc.scalar.dma_start(bih2_sb[s][:], bih2_d[s].ap())
                # weight loads in consumption order across BOTH rings: all of
                # side l first (level 0 starts sooner), then side r, then the
                # projection weights split across the rings behind them.
                for s in "lr":
                    wt_sb[s] = []
                    for k in range(KT):
                        t = wtp.tile([128, 3 * H], FP8, tag=f"wt{s}{k}")
                        eng = nc.sync if k % 2 == 0 else nc.scalar
                        eng.dma_start(t[:], wt_d[s].ap()[128 * k:128 * (k + 1), :])
                        wt_sb[s].append(t)
                Vh = Vs // 2
                nc.sync.dma_start(wo_sb[:, :, :, :Vh], woT_d.ap()[:, :, :, :Vh])
                nc.scalar.dma_start(wo_sb[:, :, :, Vh:], woT_d.ap()[:, :, :, Vh:])
                nc.sync.dma_start(bo_sb[:], bo_d.ap())

                n = Bl
                for lvl in range(DEPTH):
                    last = lvl == DEPTH - 1
                    # last level: side-major layout [128, side, KT, n] so each
                    # side's leaves are 1KB-contiguous per partition and can
                    # be bounced to DRAM (and AllGathered) as soon as that
                    # side finishes.
                    if last:
                        nxt = stp.tile([128, 2, KT, n], AGDT, tag="st8",
                                       name=f"nxt{lvl}", bufs=1)
                    else:
                        nxt = stp.tile([128, KT, 2 * n], BF16, tag="st",
                                       name=f"nxt{lvl}")
                    for si, s in enumerate("lr"):
                        for sl in range(NSL):
                            ko0 = sl * SG
                            ps = {}
                            for gi, mb in (("r", ko0), ("z", KT + ko0), ("g", 2 * KT + ko0)):
                                p = pst.tile([128, SG, n], F32, tag="ps")
                                for mj in range(SG):
                                    m = mb + mj
                                    for k in range(KT):
                                        nc.tensor.matmul(
                                            p[:, mj, :],
                                            wt_sb[s][k][:, 128 * m:128 * (m + 1)],
                                            cur[:, k, :n],
                                            start=(k == 0), stop=(k == KT - 1))
                                ps[gi] = p
                            # biases folded in via free-dim-broadcast adds (DVE)
                            def _bias(mb_):
                                return wb_sb[s][:, mb_:mb_ + SG].unsqueeze(2)                                    .broadcast_to((128, SG, n))
                            y_r = gp.tile([128, SG, n], F32, tag="yr")
                            nc.vector.tensor_tensor(y_r[:], ps["r"][:], _bias(ko0), op=ALU.add)
                            r_t = gp.tile([128, SG, n], F32, tag="r")
                            nc.scalar.activation(r_t[:], y_r[:], AF.Sigmoid)
                            y_z = gp.tile([128, SG, n], F32, tag="yz")
                            nc.vector.tensor_tensor(y_z[:], ps["z"][:], _bias(KT + ko0), op=ALU.add)
                            z_t = gp.tile([128, SG, n], F32, tag="z")
                            nc.scalar.activation(z_t[:], y_z[:], AF.Sigmoid)
                            y_g = gp.tile([128, SG, n], F32, tag="yg")
                            nc.vector.tensor_tensor(y_g[:], ps["g"][:], _bias(2 * KT + ko0), op=ALU.add)
                            t_t = gp.tile([128, SG, n], F32, tag="t")
                            nc.vector.tensor_tensor(t_t[:], y_g[:], r_t[:], op=ALU.mult)
                            nc.vector.tensor_tensor(
                                t_t[:], t_t[:],
                                bih2_sb[s][:, ko0:ko0 + SG].unsqueeze(2)
                                .broadcast_to((128, SG, n)), op=ALU.add)
                            n_t = gp.tile([128, SG, n], F32, tag="n")
                            nc.scalar.activation(n_t[:], t_t[:], AF.Tanh)
                            u_t = gp.tile([128, SG, n], F32, tag="u")
                            nc.vector.scalar_tensor_tensor(
                                u_t[:], n_t[:], -1.0, cur[:, ko0:ko0 + SG, :n],
                                op0=ALU.mult, op1=ALU.add)  # u = h - n
                            nc.vector.tensor_tensor(u_t[:], u_t[:], z_t[:], op=ALU.mult)
                            dst = (nxt[:, si, ko0:ko0 + SG, :] if last else
                                   nxt[:, ko0:ko0 + SG, si * n:si * n + n])
                            nc.vector.tensor_tensor(dst, u_t[:], n_t[:],
                                                    op=ALU.add)
                        if last:
                            # this side's leaves are complete: bounce to DRAM
                            # so its AllGather can start under remaining work
                            eng = nc.sync if si == 0 else nc.scalar
                            eng.dma_start(
                                lvb[si][:],
                                nxt[:, si].rearrange("p k j -> p (k j)"))
                    cur = nxt
                    n *= 2

        # ---------------- leaves all-gather (two overlapped halves) --------
        with nc.named_scope("ag_leaves"):
            for si in range(2):
                nc.gpsimd.collective_compute(
                    "AllGather", ALU.bypass, replica_groups=rg,
                    ins=[lvb[si].opt()], outs=[ag_half[si].ap()])

        # ---------------- projection + log-softmax ----------------
        with nc.named_scope("proj"):
            with tc.tile_pool(name="leaves", bufs=1) as lvp, \
                 tc.tile_pool(name="logits", bufs=3) as lgp, \
                 tc.tile_pool(name="scr", bufs=4) as scp, \
                 tc.tile_pool(name="stats", bufs=2) as sp2, \
                 tc.tile_pool(name="psproj", bufs=8, space="PSUM") as psp:
                # lvbig[p, c, si, k, j]: one contiguous 1KB-per-partition DMA
                # per (source core, side); side-0 chunks first so the first
                # row tiles can start while AG #1 is still in flight.
                lvbig = lvp.tile([128, N_CORES, 2, KT, 128], AGDT, tag="lvbig")
                for si in range(2):
                    for c in range(N_CORES):
                        eng = nc.sync if c % 2 == 0 else nc.scalar
                        eng.dma_start(
                            lvbig[:, c, si],
                            ag_half[si].ap()[c].rearrange(
                                "p (k j) -> p k j", k=KT))

                # unnormalized logits stream out as soon as each row tile is
                # done; the per-shard softmax denominators are returned as a
                # tiny second output and log(sum) is folded into the host-side
                # unshard pass.
                s_all = sp2.tile([128, RT], F32, tag="sall", name="s_all")
                # side-0 row tiles first: they only need AG #0, so they run
                # while AG #1 is still in flight
                for r in list(range(0, RT, 2)) + list(range(1, RT, 2)):
                    lg = lgp.tile([128, Vs], BF16, tag="lg", name=f"lg{r}")
                    sp = sp2.tile([128, NCH], F32, tag="spart", name=f"sp{r}")
                    # k-outer so one stationary (leaves) tile serves all NCH
                    # chunks; each chunk accumulates in its own PSUM bank
                    pps = [psp.tile([128, CW], F32, tag="pp", name=f"pp{r}_{nch}")
                           for nch in range(NCH)]
                    for k2 in range(KT2):
                        lhsT = lvbig[:, r // 2, r % 2, 2 * k2:2 * k2 + 2, :]
                        for nch in range(NCH):
                            nc.tensor.matmul(
                                pps[nch][:], lhsT,
                                wo_sb[:, k2, :, CW * nch:CW * (nch + 1)],
                                perf_mode=mybir.MatmulPerfMode.DoubleRow,
                                start=(k2 == 0), stop=(k2 == KT2 - 1))
                    for nch in range(NCH):
                        # bias add fused into the PSUM->SBUF copy
                        nc.vector.tensor_tensor(
                            lg[:, CW * nch:CW * (nch + 1)], pps[nch][:],
                            bo_sb[:, CW * nch:CW * (nch + 1)],
                            op=ALU.add)
                        ex = scp.tile([128, CW], BF16, tag="exp",
                                      name=f"ex{r}_{nch}")
                        nc.scalar.activation(ex[:],
                                             lg[:, CW * nch:CW * (nch + 1)],
                                             AF.Exp,
                                             accum_out=sp[:, nch:nch + 1])
                    nc.vector.reduce_sum(s_all[:, r:r + 1], sp[:],
                                         axis=mybir.AxisListType.X)
                    nc.sync.dma_start(out_d.ap()[128 * r:128 * (r + 1), :], lg[:])
                nc.scalar.dma_start(s_out_d.ap()[:, :], s_all[:])

    _dedup_ldweights(nc)
    nc.compile()
    return nc


def _dedup_ldweights(nc):
    """Remove consecutive duplicate InstLdweights from the scheduled PE
    stream: the PE keeps the stationary operand loaded across matmuls, so a
    reload of the identical weights AP between non-self-loading matmuls is
    pure overhead (tile_legalize emits one per matmul unconditionally)."""
    removed = 0
    for fn in nc.m.functions:
        for blk in fn.blocks:
            prev = None   # (ap_str, perf_mode, tile_position, is_transpose)
            keep = []
            for inst in blk.instructions:
                if isinstance(inst, mybir.InstLdweights):
                    si = inst.sync_info
                    clean = si is None or (not si.on_wait and not si.on_update)
                    key = (str(inst.ins[0]), str(inst.perf_mode),
                           str(inst.tile_position), str(inst.is_transpose))
                    if clean and key == prev:
                        removed += 1
                        continue
                    prev = key
                elif isinstance(inst, mybir.InstMatmult):
                    # non-self-loading matmuls leave the stationary intact;
                    # anything else (self-loading, transpose) invalidates it
                    if inst.ldweights is not False:
                        prev = None
                elif inst.engine == mybir.EngineType.PE and inst.is_executable():
                    prev = None
                keep.append(inst)
            if removed:
                blk.instructions[:] = keep
    return removed


_CACHE = {}


def _get(B, H, V, DEPTH):
    key = (B, H, V, DEPTH)
    if key not in _CACHE:
        _CACHE[key] = _build(B, H, V, DEPTH)
    return _CACHE[key]


def _pack_inputs(B, H, V, DEPTH, encoding, Whh_l, bih_l, bhh_l, Whh_r, bih_r,
                 bhh_r, W_out, b_out):
    """Host-side shard + transpose + cast. Returns in_maps for the 8 cores."""
    KT = H // 128
    Bl = B // N_CORES
    Vs = V // N_CORES

    KT2 = KT // 2
    woT = np.ascontiguousarray(W_out.T).astype(np.float32)    # [H, V]
    encT = np.ascontiguousarray(encoding.T).astype(BFNP)      # [H, B]

    shared = {}
    for s, Whh, bih, bhh in (("l", Whh_l, bih_l, bhh_l), ("r", Whh_r, bih_r, bhh_r)):
        shared[f"wt_{s}"] = np.ascontiguousarray(Whh.T).astype(
            mybir.dt.np(FP8))  # [H, 3H] fp8: weight-load bound, not precision bound
        # bias row folded into the matmul: sigmoid gates get bih+bhh,
        # candidate gate gets bhh only (bih_n is added after the r-multiply)
        wb = np.concatenate([(bih + bhh)[:2 * H], bhh[2 * H:]])
        shared[f"wb_{s}"] = np.ascontiguousarray(
            wb.reshape(3 * KT, 128).T.astype(np.float32))
        shared[f"bih2_{s}"] = np.ascontiguousarray(
            bih[2 * H:].reshape(KT, 128).T.astype(np.float32))  # [128, KT]

    in_maps = []
    for c in range(N_CORES):
        m = dict(shared)
        m["encT"] = np.ascontiguousarray(encT[:, c * Bl:(c + 1) * Bl])
        w = woT[:, c * Vs:(c + 1) * Vs].reshape(KT2, 2, 128, Vs)
        m["woT"] = np.ascontiguousarray(
            w.transpose(2, 0, 1, 3)).astype(mybir.dt.np(FP8))
        m["bo"] = np.ascontiguousarray(np.broadcast_to(
            b_out[c * Vs:(c + 1) * Vs].astype(BFNP), (128, Vs)))
        in_maps.append(m)
    return in_maps


def _run(B, H, V, DEPTH, inputs, trace=False, nc=None):
    if nc is None:
        nc = _get(B, H, V, DEPTH)
    in_maps = _pack_inputs(B, H, V, DEPTH, **inputs)
    res = bass_utils.run_bass_kernel_spmd(
        nc, in_maps, core_ids=list(range(N_CORES)), trace=trace)

    L = 1 << DEPTH
    Bl = B // N_CORES
    Vs = V // N_CORES
    # leaf column order per core: col = jj*Bl + e with jj = bitrev(true leaf)
    rev = np.array([int(format(t, f"0{DEPTH}b")[::-1], 2) for t in range(L)])
    # log-softmax denominator: sum the per-shard exp-sums across cores
    s_tot = np.zeros((B * L,), np.float64)
    for c in range(N_CORES):
        s = res.results[c]["s_out"]                  # [128, RT]
        s_tot += s.T.reshape(-1).astype(np.float64)  # row = rt*128 + p
    lse = np.log(s_tot).astype(np.float32)           # [B*L] in device row order
    lse = lse.reshape(N_CORES, L, Bl).transpose(0, 2, 1).reshape(B, L)[:, rev]
    full = np.empty((B, L, V), np.float32)
    for c in range(N_CORES):
        o = np.asarray(res.results[c]["out"], np.float32)   # [B*L, Vs] (bf16)
        o = o.reshape(N_CORES, L, Bl, Vs)            # [src_core, jj, e, v]
        o = o.transpose(0, 2, 1, 3).reshape(B, L, Vs)
        full[:, :, c * Vs:(c + 1) * Vs] = o[:, rev, :] - lse[:, :, None]
    return full, res


def kernel(**inputs):
    enc = np.asarray(inputs["encoding"], np.float32)
    B, H = enc.shape
    V = np.asarray(inputs["W_out"]).shape[0]
    DEPTH = int(inputs["depth"])
    args = {k: np.asarray(v, np.float32) for k, v in inputs.items() if k != "depth"}
    full, _ = _run(B, H, V, DEPTH, args)
    return full



# revision 23
# speedup vs baseline: 1.2411x; 1.1096x over previous
"""DecoderTreeRNN Trainium2 kernel (8 NeuronCores, single SPMD launch).

  - Tree expansion: data-parallel over batch B (8 examples/core). GRU states
    kept transposed [H, nodes] in bf16; each level is ghT = WhhT.T @ hT with
    fp8(e4m3) weight tiles stationary on the PE (fp8 FWL makes the weight
    load, the tree's floor, 2-4x faster; states stay bf16). Gate biases are
    folded in with free-dim-broadcast adds on VectorE; sigmoid/tanh run on
    ScalarE from one ACT table set, all on 4-wide m-tile slabs. Children are
    concatenated [left | right]; the bit-reversed leaf order is undone on
    the host during unshard. The last level writes fp8 states directly.
  - The fp8 leaf states are AllGathered so every core holds all B*32 rows.
  - Output projection: tensor-parallel over vocab (4000 columns/core),
    fp8 DoubleRow matmuls (K=256 per tile, pre-paired k=256*k2+128*j+p
    layout on both operands). Per row tile one stationary leaf tile serves
    all 8 vocab chunks, each accumulating in its own PSUM bank. The f32
    vocab bias is added during the PSUM->SBUF copy (VectorE) and exp +
    row-sum is fused on ScalarE via accum_out. Unnormalized logits stream
    straight out; each core also returns its per-row exp-sums and
    -log(sum over cores) is folded into the host-side unshard pass.
  DMA discipline: the two HWDGE rings (SP + ACT) are ordered FIFOs - small
  latency-critical inputs and right-side tree weights on the ACT ring,
  left-side tree weights then projection weights on the SP ring.
"""

import sys

for _p in ("/opt/trn_rl_repo",):
    if _p not in sys.path:
        sys.path.append(_p)

import numpy as np
import ml_dtypes

import concourse.bass as bass
from concourse import bacc, tile, mybir
from concourse import bass_utils
from concourse.tile_rust import add_dep_helper
from contextlib import ExitStack

BF16 = mybir.dt.bfloat16
F32 = mybir.dt.float32
AF = mybir.ActivationFunctionType
ALU = mybir.AluOpType
BFNP = ml_dtypes.bfloat16
FP8 = mybir.dt.float8e4
FP8_AG = True   # leaves in fp8: feeds the DoubleRow projection

N_CORES = 8
CW = 500  # vocab chunk width (<=512 fp32 psum bank)


def _build(B, H, V, DEPTH):
    KT = H // 128            # contraction tiles
    MT = 3 * KT              # output m-tiles per GRU side
    Bl = B // N_CORES        # examples per core
    L = 1 << DEPTH           # leaves per example
    NLOC = Bl * L            # local leaf count
    ROWS = B * L             # total leaf rows
    RT = ROWS // 128         # row tiles
    Vs = V // N_CORES        # vocab shard
    NCH = Vs // CW           # chunks per shard
    SG = min(4, KT)          # m-tiles per gate slab
    NSL = KT // SG           # slabs per gate
    assert B % N_CORES == 0 and H % 128 == 0 and V % N_CORES == 0
    assert Vs % CW == 0 and ROWS % 128 == 0 and RT <= 512
    assert SG * 128 <= 512  # psum slab fits one bank

    nc = bacc.Bacc("TRN2", target_bir_lowering=False, debug=False,
                   num_devices=N_CORES, dynamic_dma_scratch_size=2048)

    # ---------------- DRAM I/O ----------------
    encT = nc.dram_tensor("encT", [H, Bl], BF16, kind="ExternalInput")
    wt_d, wb_d, bih2_d = {}, {}, {}
    for s in "lr":
        wt_d[s] = nc.dram_tensor(f"wt_{s}", [H, 3 * H], FP8, kind="ExternalInput")
        wb_d[s] = nc.dram_tensor(f"wb_{s}", [128, 3 * KT], F32, kind="ExternalInput")
        bih2_d[s] = nc.dram_tensor(f"bih2_{s}", [128, KT], F32,
                                   kind="ExternalInput")
    KT2 = KT // 2            # DoubleRow k-tiles (K=256 each)
    woT_d = nc.dram_tensor("woT", [128, KT2, 2, Vs], FP8, kind="ExternalInput")
    bo_d = nc.dram_tensor("bo", [128, Vs], BF16, kind="ExternalInput")
    out_d = nc.dram_tensor("out", [ROWS, Vs], BF16, kind="ExternalOutput")

    AGDT = FP8 if FP8_AG else BF16
    # leaves are exchanged in two halves (side l, then side r) so the first
    # AllGather overlaps the second half of level-4 compute and the second
    # overlaps projection on the first half's rows. Layout [128, KT*128]
    # keeps every DMA 1KB-contiguous per partition.
    ag_half = [nc.dram_tensor(f"ag_lv{si}", [N_CORES, 128, KT * 128], AGDT,
                              kind="Internal", addr_space="Shared")
               for si in range(2)]
    warm_in = nc.dram_tensor("warm_in", [1, 1], mybir.dt.uint8, kind="Internal")
    warm_out = nc.dram_tensor("warm_out", [N_CORES, 1], mybir.dt.uint8,
                              kind="Internal", addr_space="Shared")
    s_out_d = nc.dram_tensor("s_out", [128, RT], F32, kind="ExternalOutput")

    rg = [list(range(N_CORES))]

    with tile.TileContext(nc) as tc, ExitStack() as ctx:
        dram = ctx.enter_context(tc.tile_pool(name="dram", bufs=1, space="DRAM"))
        wproj = ctx.enter_context(tc.tile_pool(name="wproj", bufs=1))
        cpool = ctx.enter_context(tc.tile_pool(name="const", bufs=1))

        # projection weights: resident for the whole kernel. Tiles are
        # allocated up front but their DMAs are issued after the tree weight
        # DMAs (below) so the tree isn't starved of HBM bandwidth at start.
        wo_sb = wproj.tile([128, KT2, 2, Vs], FP8, tag="wo8", name="wo8")
        bo_sb = cpool.tile([128, Vs], BF16, tag="bo")
        ones_sb = cpool.tile([1, 128], BF16, tag="ones")
        nc.vector.memset(ones_sb[:], 1.0)

        lvb = [dram.tile([128, KT * 128], AGDT, tag=f"lvb{si}", name=f"lvb{si}")
               for si in range(2)]

        # warmup collective: pays the ~11us ncfw mesh-setup cost under the
        # tree so the real leaf AllGathers start promptly
        nc.gpsimd.collective_compute(
            "AllGather", ALU.bypass, replica_groups=rg,
            ins=[warm_in.ap()], outs=[warm_out.ap()])

        # ---------------- tree expansion ----------------
        with nc.named_scope("tree"):
            with tc.tile_pool(name="wtree", bufs=1) as wtp, \
                 tc.tile_pool(name="state", bufs=2) as stp, \
                 tc.tile_pool(name="gates", bufs=2) as gp, \
                 tc.tile_pool(name="pstree", bufs=8, space="PSUM") as pst:
                # latency-critical small inputs go on the ACT HWDGE ring so
                # they aren't stuck behind the big weight loads (SP ring FIFO)
                cur = stp.tile([128, KT, Bl], BF16, tag="st")
                nc.scalar.dma_start(cur[:], encT.ap().rearrange("(k p) b -> p k b", k=KT))
                wt_sb, wb_sb, bih2_sb = {}, {}, {}
                for s in "lr":
                    wb_sb[s] = wtp.tile([128, 3 * KT], F32, tag=f"wb{s}", name=f"wb_sb_{s}")
                    nc.scalar.dma_start(wb_sb[s][:], wb_d[s].ap())
                    bih2_sb[s] = wtp.tile([128, KT], F32, tag=f"bi{s}", name=f"bih2_sb_{s}")
                    nc.scalar.dma_start(bih2_sb[s][:], bih2_d[s].ap())
                # weight loads in consumption order across BOTH rings: all of
                # side l first (level 0 starts sooner), then side r, then the
                # projection weights split across the rings behind them.
                for s in "lr":
                    wt_sb[s] = []
                    for k in range(KT):
                        t = wtp.tile([128, 3 * H], FP8, tag=f"wt{s}{k}")
                        eng = nc.sync if k % 2 == 0 else nc.scalar
                        eng.dma_start(t[:], wt_d[s].ap()[128 * k:128 * (k + 1), :])
                        wt_sb[s].append(t)
                Vh = Vs // 2
                nc.sync.dma_start(wo_sb[:, :, :, :Vh], woT_d.ap()[:, :, :, :Vh])
                nc.scalar.dma_start(wo_sb[:, :, :, Vh:], woT_d.ap()[:, :, :, Vh:])
                nc.sync.dma_start(bo_sb[:], bo_d.ap())

                n = Bl
                for lvl in range(DEPTH):
                    last = lvl == DEPTH - 1
                    # last level: one tile PER SIDE so the side-l bounce DMA
                    # (and its AllGather) fire as soon as side l finishes,
                    # overlapping side r's compute.
                    if last:
                        nxt_side = [
                            stp.tile([128, KT, n], AGDT, tag=f"st8{si}",
                                     name=f"leaf{si}", bufs=1)
                            for si in range(2)]
                    else:
                        nxt = stp.tile([128, KT, 2 * n], BF16, tag="st",
                                       name=f"nxt{lvl}")
                    for si, s in enumerate("lr"):
                        for sl in range(NSL):
                            ko0 = sl * SG
                            ps = {}
                            for gi, mb in (("r", ko0), ("z", KT + ko0), ("g", 2 * KT + ko0)):
                                p = pst.tile([128, SG, n], F32, tag="ps")
                                for mj in range(SG):
                                    m = mb + mj
                                    for k in range(KT):
                                        nc.tensor.matmul(
                                            p[:, mj, :],
                                            wt_sb[s][k][:, 128 * m:128 * (m + 1)],
                                            cur[:, k, :n],
                                            start=(k == 0), stop=(k == KT - 1))
                                ps[gi] = p
                            # biases folded in via free-dim-broadcast adds (DVE)
                            def _bias(mb_):
                                return wb_sb[s][:, mb_:mb_ + SG].unsqueeze(2)                                    .broadcast_to((128, SG, n))
                            y_r = gp.tile([128, SG, n], F32, tag="yr")
                            nc.vector.tensor_tensor(y_r[:], ps["r"][:], _bias(ko0), op=ALU.add)
                            r_t = gp.tile([128, SG, n], F32, tag="r")
                            nc.scalar.activation(r_t[:], y_r[:], AF.Sigmoid)
                            y_z = gp.tile([128, SG, n], F32, tag="yz")
                            nc.vector.tensor_tensor(y_z[:], ps["z"][:], _bias(KT + ko0), op=ALU.add)
                            z_t = gp.tile([128, SG, n], F32, tag="z")
                            nc.scalar.activation(z_t[:], y_z[:], AF.Sigmoid)
                            y_g = gp.tile([128, SG, n], F32, tag="yg")
                            nc.vector.tensor_tensor(y_g[:], ps["g"][:], _bias(2 * KT + ko0), op=ALU.add)
                            t_t = gp.tile([128, SG, n], F32, tag="t")
                            nc.vector.tensor_tensor(t_t[:], y_g[:], r_t[:], op=ALU.mult)
                            nc.vector.tensor_tensor(
                                t_t[:], t_t[:],
                                bih2_sb[s][:, ko0:ko0 + SG].unsqueeze(2)
                                .broadcast_to((128, SG, n)), op=ALU.add)
                            n_t = gp.tile([128, SG, n], F32, tag="n")
                            nc.scalar.activation(n_t[:], t_t[:], AF.Tanh)
                            u_t = gp.tile([128, SG, n], F32, tag="u")
                            nc.vector.scalar_tensor_tensor(
                                u_t[:], n_t[:], -1.0, cur[:, ko0:ko0 + SG, :n],
                                op0=ALU.mult, op1=ALU.add)  # u = h - n
                            nc.vector.tensor_tensor(u_t[:], u_t[:], z_t[:], op=ALU.mult)
                            dst = (nxt_side[si][:, ko0:ko0 + SG, :] if last
                                   else nxt[:, ko0:ko0 + SG, si * n:si * n + n])
                            nc.vector.tensor_tensor(dst, u_t[:], n_t[:],
                                                    op=ALU.add)
                        if last:
                            # this side's leaves are complete: bounce to DRAM
                            # so its AllGather can start under remaining work
                            eng = nc.sync if si == 0 else nc.scalar
                            eng.dma_start(
                                lvb[si][:],
                                nxt_side[si][:].rearrange("p k j -> p (k j)"))
                    if not last:
                        cur = nxt
                        n *= 2

        # ---------------- leaves all-gather (two overlapped halves) --------
        with nc.named_scope("ag_leaves"):
            for si in range(2):
                nc.gpsimd.collective_compute(
                    "AllGather", ALU.bypass, replica_groups=rg,
                    ins=[lvb[si].opt()], outs=[ag_half[si].ap()])

        # ---------------- projection + log-softmax ----------------
        with nc.named_scope("proj"):
            with tc.tile_pool(name="leaves", bufs=1) as lvp, \
                 tc.tile_pool(name="logits", bufs=3) as lgp, \
                 tc.tile_pool(name="scr", bufs=4) as scp, \
                 tc.tile_pool(name="stats", bufs=2) as sp2, \
                 tc.tile_pool(name="psproj", bufs=8, space="PSUM") as psp:
                # lvbig[p, c, si, k, j]: one contiguous 1KB-per-partition DMA
                # per (source core, side); side-0 chunks first so the first
                # row tiles can start while AG #1 is still in flight.
                lvbig = lvp.tile([128, N_CORES, 2, KT, 128], AGDT, tag="lvbig")
                for si in range(2):
                    for c in range(N_CORES):
                        eng = nc.sync if c % 2 == 0 else nc.scalar
                        eng.dma_start(
                            lvbig[:, c, si],
                            ag_half[si].ap()[c].rearrange(
                                "p (k j) -> p k j", k=KT))

                # unnormalized logits stream out as soon as each row tile is
                # done; the per-shard softmax denominators are returned as a
                # tiny second output and log(sum) is folded into the host-side
                # unshard pass.
                s_all = sp2.tile([128, RT], F32, tag="sall", name="s_all")
                # side-0 row tiles first: they only need AG #0, so they run
                # while AG #1 is still in flight
                for r in list(range(0, RT, 2)) + list(range(1, RT, 2)):
                    lg = lgp.tile([128, Vs], BF16, tag="lg", name=f"lg{r}")
                    sp = sp2.tile([128, NCH], F32, tag="spart", name=f"sp{r}")
                    # k-outer so one stationary (leaves) tile serves all NCH
                    # chunks; each chunk accumulates in its own PSUM bank
                    pps = [psp.tile([128, CW], F32, tag="pp", name=f"pp{r}_{nch}")
                           for nch in range(NCH)]
                    for k2 in range(KT2):
                        lhsT = lvbig[:, r // 2, r % 2, 2 * k2:2 * k2 + 2, :]
                        for nch in range(NCH):
                            nc.tensor.matmul(
                                pps[nch][:], lhsT,
                                wo_sb[:, k2, :, CW * nch:CW * (nch + 1)],
                                perf_mode=mybir.MatmulPerfMode.DoubleRow,
                                start=(k2 == 0), stop=(k2 == KT2 - 1))
                    for nch in range(NCH):
                        # bias add fused into the PSUM->SBUF copy
                        nc.vector.tensor_tensor(
                            lg[:, CW * nch:CW * (nch + 1)], pps[nch][:],
                            bo_sb[:, CW * nch:CW * (nch + 1)],
                            op=ALU.add)
                        ex = scp.tile([128, CW], BF16, tag="exp",
                                      name=f"ex{r}_{nch}")
                        nc.scalar.activation(ex[:],
                                             lg[:, CW * nch:CW * (nch + 1)],
                                             AF.Exp,
                                             accum_out=sp[:, nch:nch + 1])
                    nc.vector.reduce_sum(s_all[:, r:r + 1], sp[:],
                                         axis=mybir.AxisListType.X)
                    nc.sync.dma_start(out_d.ap()[128 * r:128 * (r + 1), :], lg[:])
                nc.scalar.dma_start(s_out_d.ap()[:, :], s_all[:])

    _dedup_ldweights(nc)
    nc.compile()
    return nc


def _dedup_ldweights(nc):
    """Remove consecutive duplicate InstLdweights from the scheduled PE
    stream: the PE keeps the stationary operand loaded across matmuls, so a
    reload of the identical weights AP between non-self-loading matmuls is
    pure overhead (tile_legalize emits one per matmul unconditionally)."""
    removed = 0
    for fn in nc.m.functions:
        for blk in fn.blocks:
            prev = None   # (ap_str, perf_mode, tile_position, is_transpose)
            keep = []
            for inst in blk.instructions:
                if isinstance(inst, mybir.InstLdweights):
                    si = inst.sync_info
                    clean = si is None or (not si.on_wait and not si.on_update)
                    key = (str(inst.ins[0]), str(inst.perf_mode),
                           str(inst.tile_position), str(inst.is_transpose))
                    if clean and key == prev:
                        removed += 1
                        continue
                    prev = key
                elif isinstance(inst, mybir.InstMatmult):
                    # non-self-loading matmuls leave the stationary intact;
                    # anything else (self-loading, transpose) invalidates it
                    if inst.ldweights is not False:
                        prev = None
                elif inst.engine == mybir.EngineType.PE and inst.is_executable():
                    prev = None
                keep.append(inst)
            if removed:
                blk.instructions[:] = keep
    return removed


_CACHE = {}


def _get(B, H, V, DEPTH):
    key = (B, H, V, DEPTH)
    if key not in _CACHE:
        _CACHE[key] = _build(B, H, V, DEPTH)
    return _CACHE[key]


def _pack_inputs(B, H, V, DEPTH, encoding, Whh_l, bih_l, bhh_l, Whh_r, bih_r,
                 bhh_r, W_out, b_out):
    """Host-side shard + transpose + cast. Returns in_maps for the 8 cores."""
    KT = H // 128
    Bl = B // N_CORES
    Vs = V // N_CORES

    KT2 = KT // 2
    woT = np.ascontiguousarray(W_out.T).astype(np.float32)    # [H, V]
    encT = np.ascontiguousarray(encoding.T).astype(BFNP)      # [H, B]

    shared = {}
    for s, Whh, bih, bhh in (("l", Whh_l, bih_l, bhh_l), ("r", Whh_r, bih_r, bhh_r)):
        shared[f"wt_{s}"] = np.ascontiguousarray(Whh.T).astype(
            mybir.dt.np(FP8))  # [H, 3H] fp8: weight-load bound, not precision bound
        # bias row folded into the matmul: sigmoid gates get bih+bhh,
        # candidate gate gets bhh only (bih_n is added after the r-multiply)
        wb = np.concatenate([(bih + bhh)[:2 * H], bhh[2 * H:]])
        shared[f"wb_{s}"] = np.ascontiguousarray(
            wb.reshape(3 * KT, 128).T.astype(np.float32))
        shared[f"bih2_{s}"] = np.ascontiguousarray(
            bih[2 * H:].reshape(KT, 128).T.astype(np.float32))  # [128, KT]

    in_maps = []
    for c in range(N_CORES):
        m = dict(shared)
        m["encT"] = np.ascontiguousarray(encT[:, c * Bl:(c + 1) * Bl])
        w = woT[:, c * Vs:(c + 1) * Vs].reshape(KT2, 2, 128, Vs)
        m["woT"] = np.ascontiguousarray(
            w.transpose(2, 0, 1, 3)).astype(mybir.dt.np(FP8))
        m["bo"] = np.ascontiguousarray(np.broadcast_to(
            b_out[c * Vs:(c + 1) * Vs].astype(BFNP), (128, Vs)))
        in_maps.append(m)
    return in_maps


def _run(B, H, V, DEPTH, inputs, trace=False, nc=None):
    if nc is None:
        nc = _get(B, H, V, DEPTH)
    in_maps = _pack_inputs(B, H, V, DEPTH, **inputs)
    res = bass_utils.run_bass_kernel_spmd(
        nc, in_maps, core_ids=list(range(N_CORES)), trace=trace)

    L = 1 << DEPTH
    Bl = B // N_CORES
    Vs = V // N_CORES
    # leaf column order per core: col = jj*Bl + e with jj = bitrev(true leaf)
    rev = np.array([int(format(t, f"0{DEPTH}b")[::-1], 2) for t in range(L)])
    # log-softmax denominator: sum the per-shard exp-sums across cores
    s_tot = np.zeros((B * L,), np.float64)
    for c in range(N_CORES):
        s = res.results[c]["s_out"]                  # [128, RT]
        s_tot += s.T.reshape(-1).astype(np.float64)  # row = rt*128 + p
    lse = np.log(s_tot).astype(np.float32)           # [B*L] in device row order
    lse = lse.reshape(N_CORES, L, Bl).transpose(0, 2, 1).reshape(B, L)[:, rev]
    full = np.empty((B, L, V), np.float32)
    for c in range(N_CORES):
        o = np.asarray(res.results[c]["out"], np.float32)   # [B*L, Vs] (bf16)
        o = o.reshape(N_CORES, L, Bl, Vs)            # [src_core, jj, e, v]
        o = o.transpose(0, 2, 1, 3).reshape(B, L, Vs)
        full[:, :, c * Vs:(c + 1) * Vs] = o[:, rev, :] - lse[:, :, None]
    return full, res


def kernel(**inputs):
    enc = np.asarray(inputs["encoding"], np.float32)
    B, H = enc.shape
    V = np.asarray(inputs["W_out"]).shape[0]
    DEPTH = int(inputs["depth"])
    args = {k: np.asarray(v, np.float32) for k, v in inputs.items() if k != "depth"}
    full, _ = _run(B, H, V, DEPTH, args)
    return full

